# revision 4
# baseline (speedup 1.0000x reference)
"""Trainium2 Bass kernel for nn_CrossAttention (B=8, S=2048, D=512, fp32).

Sharding: data-parallel over batch across the 8 NeuronCores (one batch
element per core); the 512x512 projection weights are replicated.

v2 — restructured for DMA/queue parallelism and PE density:
  * Inputs split across all three DMA queues (sync HWDGE, scalar HWDGE,
    gpsimd SWDGE); weights and some late z chunks use gpsimd cast-DMA
    (fp32 DRAM -> bf16 SBUF in flight).
  * z_k / z_q tile-0 / W are transposed on the PE (startup, PE otherwise
    idle); z_v and z_q tiles 1-3 are transposed by the DMA XBAR
    (dma_start(transpose=True)) off the critical PE path.
  * K-projection and scores tile 0 chase the z_k chunk DMAs.
  * LayerNorm epilogue folds the softmax row-sum into the LN scale:
      out = (AV - mean(AV)) * s * gamma + beta,  s = 1/sqrt(var + eps*r^2)
    so AV/rowsum never materializes; bn_stats reads PSUM directly;
    gamma/beta passes run on gpsimd; outputs go out on sync (gpsimd for
    tile 0 while sync drains transposes).
  Softmax skips the max-subtraction: scores ~ N(0,1), so exp() is safely
  within fp32 range; matches jax softmax up to rounding.
"""

import math
import os
import sys
from contextlib import ExitStack

for _p in ("/opt/trn_rl_repo", "/root/.axon_site/_ro/trn_rl_repo"):
    if os.path.isdir(_p) and _p not in sys.path:
        sys.path.append(_p)

import numpy as np

import concourse.bacc as bacc
import concourse.bass as bass
import concourse.mybir as mybir
import concourse.tile as tile
from concourse.bass import ds, ts
from concourse.bass_utils import run_bass_kernel_spmd
from concourse.masks import make_identity

P = 128
B = 8
S = 2048
D = 512
DC = D // P       # 4   chunks of the model dim
SC = S // P       # 16  chunks of the sequence dim
NQ = 512          # sq macro-tile width (matmul free dim)
NT = S // NQ      # 4   macro tiles
GPC = 4           # 128-row groups per DMA chunk (1 MB fp32)
NCK = SC // GPC   # 4   chunks per z tensor
NM = NQ // P      # 4   subtiles per macro tile
LN_EPS = 1e-5
F32 = mybir.dt.float32
BF16 = mybir.dt.bfloat16
ADD = mybir.AluOpType.add
MUL = mybir.AluOpType.mult

INPUT_NAMES = (
    "z_q", "z_k", "z_v", "Wq", "bq", "Wk", "bk", "Wv", "bv",
    "ln_gamma", "ln_beta",
)


def _bcast_row_load(nc, dst, src_1d):
    """DMA-replicate a [D] DRAM vector across all partitions of dst [P, D]."""
    src = bass.AP(
        tensor=src_1d.tensor,
        offset=src_1d.offset,
        ap=[[0, dst.shape[0]]] + list(src_1d.ap),
    )
    nc.gpsimd.dma_start(out=dst, in_=src)


def _build_tile_kernel(tc, ins, out):
    nc = tc.nc
    z_q, z_k, z_v, Wq, bq, Wk, bk, Wv, bv, ln_g, ln_b = (ins[k] for k in INPUT_NAMES)
    inv_sqrt_d = 1.0 / math.sqrt(D)
    outr = out.rearrange("(so p) d -> p so d", p=P)
    zr_k = z_k.rearrange("(g p) d -> p g d", p=P)
    zr_v = z_v.rearrange("(g p) d -> p g d", p=P)
    zr_q = z_q.rearrange("(g p) d -> p g d", p=P)

    ctx = ExitStack()
    singles = ctx.enter_context(tc.tile_pool(name="singles", bufs=1))

    # ---- persistent SBUF ----
    ident = singles.tile([P, P], F32)
    make_identity(nc, ident)
    ident16 = singles.tile([P, P], BF16)
    nc.vector.tensor_copy(ident16, ident)

    Kt = singles.tile([P, DC, S], BF16)     # [e_in, e_out, sk]
    V = singles.tile([P, SC, D], BF16)      # [sk_in, sk_out, e]
    zt_v = singles.tile([P, DC, S], BF16)   # [d_in, d_out, sk]
    zt_k = singles.tile([P, DC, S], BF16)
    wt = {k: singles.tile([P, DC, D], BF16, name=f"wt_{k}") for k in "kqv"}

    ones_f32 = singles.tile([P, 2], F32)
    nc.vector.memset(ones_f32, 1.0)
    ones = singles.tile([P, 2], BF16)
    nc.vector.tensor_copy(ones, ones_f32)
    bq_sb = singles.tile([P, DC], F32)
    bk_sb = singles.tile([P, DC], F32)
    bv_sb = singles.tile([P, D], F32)
    gam_sb = singles.tile([P, D], F32)
    bet_sb = singles.tile([P, D], F32)
    stats = singles.tile([P, SC, 2], F32)   # per-subtile (mean, var) of AV
    tvals = singles.tile([P, SC], F32)      # var + eps*r^2
    svals = singles.tile([P, SC], F32)      # 1/sqrt(tvals)
    nms = singles.tile([P, SC], F32)        # -mean*s

    # ---- pools ----
    wp = ctx.enter_context(tc.tile_pool(name="w16", bufs=2))
    znat = ctx.enter_context(tc.tile_pool(name="znat", bufs=3))
    z16p = ctx.enter_context(tc.tile_pool(name="z16", bufs=3))
    z16l = ctx.enter_context(tc.tile_pool(name="z16l", bufs=1))   # long-lived bf16
    qtp = ctx.enter_context(tc.tile_pool(name="qt", bufs=2))
    ztqp = ctx.enter_context(tc.tile_pool(name="ztq", bufs=2))
    expp = ctx.enter_context(tc.tile_pool(name="expp", bufs=2))
    otp = ctx.enter_context(tc.tile_pool(name="otp", bufs=6))
    ep = ctx.enter_context(tc.tile_pool(name="ep", bufs=4))
    ps_tp = ctx.enter_context(tc.tile_pool(name="ps_tp", bufs=2, space="PSUM"))
    ps_sc = ctx.enter_context(tc.tile_pool(name="ps_sc", bufs=2, space="PSUM"))
    ps_av = ctx.enter_context(tc.tile_pool(name="ps_av", bufs=3, space="PSUM"))
    ps_rs = ctx.enter_context(tc.tile_pool(name="ps_rs", bufs=1, space="PSUM"))

    # ---- gpsimd queue: biases first, then weight cast-DMAs, then late z ----
    nc.gpsimd.dma_start(bq_sb, bq.rearrange("(eo p) -> p eo", p=P))
    nc.gpsimd.dma_start(bk_sb, bk.rearrange("(eo p) -> p eo", p=P))
    _bcast_row_load(nc, bv_sb, bv)
    _bcast_row_load(nc, gam_sb, ln_g)
    _bcast_row_load(nc, bet_sb, ln_b)
    w16 = {}
    for k, W in (("k", Wk), ("q", Wq), ("v", Wv)):
        w16[k] = wp.tile([P, DC, D], BF16, tag="w16", name=f"w16_{k}")
        nc.gpsimd.dma_start(w16[k], W.rearrange("(eo p) d -> p eo d", p=P))
    zv16_3 = z16l.tile([P, GPC, D], BF16, name="zv16_3")
    nc.gpsimd.dma_start(zv16_3, zr_v[:, ds(3 * GPC, GPC), :])
    zq16_1 = z16l.tile([P, GPC, D], BF16, name="zq16_1")
    nc.gpsimd.dma_start(zq16_1, zr_q[:, ds(GPC, GPC), :])

    # ---- sync + scalar queues: fp32 chunk loads (program order = queue order) ----
    def nat_tile(name):
        return znat.tile([P, GPC, D], F32, tag="znat", name=name)

    zk_nat = [nat_tile(f"zk_nat{c}") for c in range(2)]
    nc.sync.dma_start(zk_nat[0], zr_k[:, ds(0, GPC), :])
    nc.sync.dma_start(zk_nat[1], zr_k[:, ds(GPC, GPC), :])
    zq0_nat = nat_tile("zq0_nat")
    nc.scalar.dma_start(zq0_nat, zr_q[:, ds(0, GPC), :])
    zk_nat.append(nat_tile("zk_nat2"))
    nc.scalar.dma_start(zk_nat[2], zr_k[:, ds(2 * GPC, GPC), :])
    zk_nat.append(nat_tile("zk_nat3"))
    nc.scalar.dma_start(zk_nat[3], zr_k[:, ds(3 * GPC, GPC), :])
    zv_nat = [nat_tile(f"zv_nat{c}") for c in range(2)]
    nc.sync.dma_start(zv_nat[0], zr_v[:, ds(0, GPC), :])
    nc.sync.dma_start(zv_nat[1], zr_v[:, ds(GPC, GPC), :])
    zv_nat.append(nat_tile("zv_nat2"))
    nc.scalar.dma_start(zv_nat[2], zr_v[:, ds(2 * GPC, GPC), :])
    zq2_nat = nat_tile("zq2_nat")
    nc.sync.dma_start(zq2_nat, zr_q[:, ds(2 * GPC, GPC), :])
    zq3_nat = nat_tile("zq3_nat")
    nc.scalar.dma_start(zq3_nat, zr_q[:, ds(3 * GPC, GPC), :])

    # ---- helpers ----
    def cast_chunk(nat, tag_name):
        t = z16p.tile([P, GPC, D], BF16, tag="z16c", name=tag_name)
        nc.vector.tensor_copy(t, nat)
        return t

    def pe_transpose_chunk(z16c, zt_dst, c):
        # transpose GPC groups x DC blocks of a bf16 chunk into zt[:, do, ...]
        for do in range(DC):
            pt = ps_tp.tile([P, GPC, P], BF16, tag="tp", name="pt")
            for j in range(GPC):
                nc.tensor.transpose(pt[:, j, :], z16c[:, j, ts(do, P)], ident16)
            nc.vector.tensor_copy(zt_dst[:, do, ds(c * GPC * P, GPC * P)], pt)

    def pe_transpose_w(key):
        # wt[d_in, do, e] from w16[e_in, eo, d]
        for do in range(DC):
            pt = ps_tp.tile([P, DC, P], BF16, tag="tp", name="pt")
            for eo in range(DC):
                nc.tensor.transpose(pt[:, eo, :], w16[key][:, eo, ts(do, P)], ident16)
            nc.vector.tensor_copy(wt[key][:, do, :], pt)

    def xbar_transpose_chunk(z16c, zt_dst, c):
        # XBAR 128x128 block transposes on the sync queue
        for do in range(DC):
            for j in range(GPC):
                g = c * GPC + j
                nc.sync.dma_start(
                    zt_dst[:, do, ts(g, P)], z16c[:, j, ts(do, P)], transpose=True
                )

    def proj_sn(zt, wtk, bias_sb, dst, sn):
        # dst[:, eo, sn-slice] = W @ z.T + b for one 512-wide s slice
        for eo in range(DC):
            ps = ps_sc.tile([P, NQ], F32, tag="sc", name="proj_ps")
            for do in range(DC):
                nc.tensor.matmul(
                    ps,
                    wtk[:, do, ts(eo, P)],
                    zt[:, do, ts(sn, NQ)],
                    start=(do == 0),
                    stop=(do == DC - 1),
                )
            nc.vector.tensor_scalar_add(
                dst[:, eo, ts(sn, NQ)], ps, bias_sb[:, eo : eo + 1]
            )

    # ---- phase 1: chase z_k; K projection; Q0; V transposes ----
    zk16 = [None] * NCK
    zk16[0] = cast_chunk(zk_nat[0], "zk16_0")
    pe_transpose_w("k")
    pe_transpose_chunk(zk16[0], zt_k, 0)
    proj_sn(zt_k, wt["k"], bk_sb, Kt, 0)
    # z_q tile 0 via PE transpose
    zq16_0 = cast_chunk(zq0_nat, "zq16_0")
    ztq0 = ztqp.tile([P, DC, NQ], BF16, tag="ztq", name="ztq0")
    pe_transpose_chunk(zq16_0, ztq0, 0)
    for c in (1, 2):
        zk16[c] = cast_chunk(zk_nat[c], f"zk16_{c}")
        pe_transpose_chunk(zk16[c], zt_k, c)
        proj_sn(zt_k, wt["k"], bk_sb, Kt, c)
    pe_transpose_w("q")
    qt0 = qtp.tile([P, DC, NQ], BF16, tag="qt", name="qt0")
    for eo in range(DC):
        ps = ps_sc.tile([P, NQ], F32, tag="sc", name="q0_ps")
        for do in range(DC):
            nc.tensor.matmul(
                ps, wt["q"][:, do, ts(eo, P)], ztq0[:, do, :],
                start=(do == 0), stop=(do == DC - 1),
            )
        nc.vector.tensor_scalar_add(qt0[:, eo, :], ps, bq_sb[:, eo : eo + 1])
    zk16[3] = cast_chunk(zk_nat[3], "zk16_3")
    pe_transpose_chunk(zk16[3], zt_k, 3)
    proj_sn(zt_k, wt["k"], bk_sb, Kt, 3)

    # z_v casts + XBAR transposes (sync queue, chase the loads)
    for c in range(3):
        zv16 = cast_chunk(zv_nat[c], f"zv16_{c}")
        xbar_transpose_chunk(zv16, zt_v, c)
    xbar_transpose_chunk(zv16_3, zt_v, 3)

    qts = {0: qt0}
    zq16s = {1: zq16_1}

    def project_q_slice(tq):
        # XBAR-transpose the z_q chunk, then project on the PE
        ztq = ztqp.tile([P, DC, NQ], BF16, tag="ztq", name=f"ztq{tq}")
        xbar_transpose_chunk(zq16s[tq], ztq, 0)
        qt = qtp.tile([P, DC, NQ], BF16, tag="qt", name=f"qt{tq}")
        for eo in range(DC):
            psq = ps_sc.tile([P, NQ], F32, tag="sc", name="psq")
            for do in range(DC):
                nc.tensor.matmul(
                    psq, wt["q"][:, do, ts(eo, P)], ztq[:, do, :],
                    start=(do == 0), stop=(do == DC - 1),
                )
            nc.vector.tensor_scalar_add(qt[:, eo, :], psq, bq_sb[:, eo : eo + 1])
        qts[tq] = qt

    def scores_tile(tq):
        expT = expp.tile([P, SC, NQ], BF16, tag="expT", name=f"expT{tq}")
        qt = qts[tq]
        for skc in range(SC):
            pss = ps_sc.tile([P, NQ], F32, tag="sc", name="pss")
            for eo in range(DC):
                nc.tensor.matmul(
                    pss, Kt[:, eo, ts(skc, P)], qt[:, eo, :],
                    start=(eo == 0), stop=(eo == DC - 1),
                )
            nc.scalar.activation(
                expT[:, skc, :], pss,
                mybir.ActivationFunctionType.Exp,
                scale=inv_sqrt_d,
            )
        return expT

    def v_projection():
        pe_transpose_w("v")
        for sko in range(SC):
            ps = ps_av.tile([P, D], F32, tag="av", name="vproj_ps")
            for do in range(DC):
                nc.tensor.matmul(
                    ps, zt_v[:, do, ts(sko, P)], wt["v"][:, do, :],
                    start=(do == 0), stop=(do == DC - 1),
                )
            nc.vector.tensor_tensor(V[:, sko, :], ps, bv_sb, ADD)

    def av_subtile(expT, tq, m):
        so = tq * NM + m
        pso = ps_av.tile([P, D], F32, tag="av", name="pso")
        psr = ps_rs.tile([P, 2], F32, tag="rs", name="psr")
        for skc in range(SC):
            lhsT = expT[:, skc, ts(m, P)]
            nc.tensor.matmul(
                pso, lhsT, V[:, skc, :], start=(skc == 0), stop=(skc == SC - 1)
            )
            nc.tensor.matmul(
                psr, lhsT, ones, start=(skc == 0), stop=(skc == SC - 1)
            )
        # stats straight off PSUM; fold rowsum into the LN scale
        st6 = ep.tile([P, 6], F32, tag="st6", name="st6")
        nc.vector.bn_stats(st6, pso)
        nc.vector.bn_aggr(stats[:, so, :], st6)
        # tvals = eps*r^2 + var (two steps: only one PSUM input per op)
        r_eps = ep.tile([P, 1], F32, tag="r_eps", name="r_eps")
        nc.vector.tensor_scalar(
            r_eps, psr[:, 0:1], float(LN_EPS), 0.0, op0=MUL, op1=ADD
        )
        nc.vector.tensor_scalar(
            tvals[:, so : so + 1], psr[:, 0:1], r_eps, stats[:, so, 1:2],
            op0=MUL, op1=ADD,
        )
        av_sb = otp.tile([P, D], F32, tag="ot", name=f"av{so}")
        nc.vector.tensor_copy(av_sb, pso)
        return av_sb

    def sqrt_batch(tq):
        msl = ds(tq * NM, NM)
        nc.scalar.activation(
            svals[:, msl], tvals[:, msl], mybir.ActivationFunctionType.Sqrt
        )
        nc.vector.reciprocal(svals[:, msl], svals[:, msl])

    def finish_subtile(av_sb, tq, m, out_eng):
        so = tq * NM + m
        nc.vector.tensor_scalar(
            nms[:, so : so + 1], stats[:, so, 0:1], svals[:, so : so + 1], -1.0,
            op0=MUL, op1=MUL,
        )
        nc.vector.tensor_scalar(
            av_sb, av_sb, svals[:, so : so + 1], nms[:, so : so + 1],
            op0=MUL, op1=ADD,
        )
        nc.gpsimd.tensor_tensor(av_sb, av_sb, gam_sb, MUL)
        nc.gpsimd.tensor_tensor(av_sb, av_sb, bet_sb, ADD)
        out_eng.dma_start(outr[:, so, :], av_sb)

    # ---- phase 2: attention body ----
    # tile 0: scores -> V projection (covers exp latency) -> AV -> Q1 -> finish
    expT = scores_tile(0)
    v_projection()
    avs = [av_subtile(expT, 0, m) for m in range(NM)]
    project_q_slice(1)
    sqrt_batch(0)
    for m in range(NM):
        finish_subtile(avs[m], 0, m, nc.gpsimd)

    for tq in range(1, NT):
        expT = scores_tile(tq)
        if tq + 1 < NT:
            # cast the fp32-loaded z_q chunk for tile tq+1 (vector), then
            # project tq+1 between scores and AV to cover the exp latency
            if tq + 1 == 2:
                zq16s[2] = cast_chunk(zq2_nat, "zq16_2")
            elif tq + 1 == 3:
                zq16s[3] = cast_chunk(zq3_nat, "zq16_3")
            project_q_slice(tq + 1)
        avs = [av_subtile(expT, tq, m) for m in range(NM)]
        sqrt_batch(tq)
        for m in range(NM):
            finish_subtile(avs[m], tq, m, nc.sync)

    ctx.close()


_NC_CACHE = None


def _build():
    global _NC_CACHE
    if _NC_CACHE is not None:
        return _NC_CACHE
    nc = bacc.Bacc("TRN2", target_bir_lowering=False, debug=False, num_devices=B)
    shapes = {
        "z_q": [S, D], "z_k": [S, D], "z_v": [S, D],
        "Wq": [D, D], "Wk": [D, D], "Wv": [D, D],
        "bq": [D], "bk": [D], "bv": [D],
        "ln_gamma": [D], "ln_beta": [D],
    }
    ins = {
        k: nc.dram_tensor(k, shapes[k], F32, kind="ExternalInput").ap()
        for k in INPUT_NAMES
    }
    out = nc.dram_tensor("out", [S, D], F32, kind="ExternalOutput").ap()
    with tile.TileContext(nc) as tc:
        _build_tile_kernel(tc, ins, out)
    nc.compile()
    _NC_CACHE = nc
    return nc


def _run(inputs, **spmd_kwargs):
    nc = _build()
    arrs = {k: np.ascontiguousarray(np.asarray(inputs[k]), dtype=np.float32)
            for k in INPUT_NAMES}
    in_maps = []
    for b in range(B):
        m = {"z_q": arrs["z_q"][b], "z_k": arrs["z_k"][b], "z_v": arrs["z_v"][b]}
        for k in ("Wq", "bq", "Wk", "bk", "Wv", "bv", "ln_gamma", "ln_beta"):
            m[k] = arrs[k]
        in_maps.append(m)
    res = run_bass_kernel_spmd(nc, in_maps, core_ids=list(range(B)), **spmd_kwargs)
    out = np.stack([res.results[b]["out"] for b in range(B)], axis=0)
    return out, res


def kernel(**inputs):
    out, _ = _run(inputs)
    return out


# revision 11
# speedup vs baseline: 1.0412x; 1.0412x over previous
"""Trainium2 Bass kernel for nn_CrossAttention (B=8, S=2048, D=512, fp32).

Sharding: data-parallel over batch across the 8 NeuronCores (one batch
element per core); the 512x512 projection weights are replicated.

v3 — DMA-bandwidth-oriented restructure:
  * All DRAM loads pack r=4 consecutive rows per partition
    ("(c p r) d -> p c (r d)") so each partition reads one contiguous
    8 KB run -> ~3-4x the per-queue HWDGE descriptor-bound rate.  The
    resulting block-permuted s/e order (x = 4*p + r) is carried
    consistently through Q/K/V, scores, and AV, and inverted for free
    by the output DMA's access pattern.
  * All z transposes and the Wk/Wq transposes run on the DMA XBAR
    (dma_start(transpose=True)) with a 3D output AP: one instruction
    transposes a [128, 512] row-block into [d, 4, 128].  The PE only
    transposes Wv (whose free dim must stay natural e for the output).
  * K-projection, Q0, and the scores-tile-0 chunks are emitted
    interleaved so the PE chases the z_k chunk arrivals.
  * LayerNorm epilogue folds the softmax row-sum into the LN scale:
      out = (AV - mean(AV)) * s * gamma + beta,  s = 1/sqrt(var + eps*r^2)
    bn_stats reads PSUM directly; gamma/beta run on gpsimd; output DMAs
    alternate between the sync and scalar queues.
  Softmax skips the max-subtraction: scores ~ N(0,1), so exp() is safely
  within fp32 range; matches jax softmax up to rounding.
"""

import math
import os
import sys
from contextlib import ExitStack

for _p in ("/opt/trn_rl_repo", "/root/.axon_site/_ro/trn_rl_repo"):
    if os.path.isdir(_p) and _p not in sys.path:
        sys.path.append(_p)

import numpy as np

import concourse.bacc as bacc
import concourse.bass as bass
import concourse.mybir as mybir
import concourse.tile as tile
from concourse.bass import ds, ts
from concourse.bass_utils import run_bass_kernel_spmd
from concourse.masks import make_identity

P = 128
B = 8
S = 2048
D = 512
DC = D // P       # 4   chunks of the model dim
SC = S // P       # 16  128-blocks of the sequence dim
NQ = 512          # sq macro-tile width (matmul free dim)
NT = S // NQ      # 4   macro tiles == z chunks
GPC = 4           # rows packed per partition (8 KB runs)
NM = NQ // P      # 4   subtiles per macro tile
LN_EPS = 1e-5
F32 = mybir.dt.float32
BF16 = mybir.dt.bfloat16
ADD = mybir.AluOpType.add
MUL = mybir.AluOpType.mult

INPUT_NAMES = (
    "z_q", "z_k", "z_v", "Wq", "bq", "Wk", "bk", "Wv", "bv",
    "ln_gamma", "ln_beta",
)


def _bcast_row_load(nc, dst, src_1d):
    """DMA-replicate a [D] DRAM vector across all partitions of dst [P, D]."""
    src = bass.AP(
        tensor=src_1d.tensor,
        offset=src_1d.offset,
        ap=[[0, dst.shape[0]]] + list(src_1d.ap),
    )
    nc.gpsimd.dma_start(out=dst, in_=src)


def _build_tile_kernel(tc, ins, out):
    nc = tc.nc
    z_q, z_k, z_v, Wq, bq, Wk, bk, Wv, bv, ln_g, ln_b = (ins[k] for k in INPUT_NAMES)
    inv_sqrt_d = 1.0 / math.sqrt(D)
    # s = c*512 + 4*p + r ; per-partition contiguous run = 4 rows = 8 KB
    outr = out.rearrange("(c p r) d -> p c r d", p=P, r=NM)
    zr_k = z_k.rearrange("(c p r) d -> p c r d", p=P, r=GPC)
    zr_v = z_v.rearrange("(c p r) d -> p c r d", p=P, r=GPC)
    zr_q = z_q.rearrange("(c p r) d -> p c r d", p=P, r=GPC)
    wr = {k: W.rearrange("(p r) d -> p r d", p=P, r=DC) for k, W in
          (("k", Wk), ("q", Wq), ("v", Wv))}

    ctx = ExitStack()
    singles = ctx.enter_context(tc.tile_pool(name="singles", bufs=1))

    # ---- persistent SBUF ----
    ident = singles.tile([P, P], F32)
    make_identity(nc, ident)
    ident16 = singles.tile([P, P], BF16)
    nc.vector.tensor_copy(ident16, ident)

    Kt = singles.tile([P, DC, S], BF16)     # [e', eo2, s']
    V = singles.tile([P, SC, D], BF16)      # [sk', sko, e]
    zt_v = singles.tile([P, DC, S], BF16)   # [d, do, s']
    zt_k = singles.tile([P, DC, S], BF16)
    wt = {k: singles.tile([P, DC, D], BF16, name=f"wt_{k}") for k in "kqv"}

    ones_f32 = singles.tile([P, 2], F32)
    nc.vector.memset(ones_f32, 1.0)
    ones = singles.tile([P, 2], BF16)
    nc.vector.tensor_copy(ones, ones_f32)
    bq_sb = singles.tile([P, DC], F32)      # b[4p + r]
    bk_sb = singles.tile([P, DC], F32)
    bv_sb = singles.tile([P, D], F32)
    gam_sb = singles.tile([P, D], F32)
    bet_sb = singles.tile([P, D], F32)
    stats = singles.tile([P, SC, 2], F32)
    tvals = singles.tile([P, SC], F32)
    svals = singles.tile([P, SC], F32)
    nms = singles.tile([P, SC], F32)

    # ---- pools ----
    znat = ctx.enter_context(tc.tile_pool(name="znat", bufs=5))
    z16p = ctx.enter_context(tc.tile_pool(name="z16", bufs=3))
    z16l = ctx.enter_context(tc.tile_pool(name="z16l", bufs=1))
    qtp = ctx.enter_context(tc.tile_pool(name="qt", bufs=2))
    ztqp = ctx.enter_context(tc.tile_pool(name="ztq", bufs=2))
    expp = ctx.enter_context(tc.tile_pool(name="expp", bufs=2))
    otp = ctx.enter_context(tc.tile_pool(name="otp", bufs=6))
    ep = ctx.enter_context(tc.tile_pool(name="ep", bufs=4))
    ps_tp = ctx.enter_context(tc.tile_pool(name="ps_tp", bufs=2, space="PSUM"))
    ps_sc = ctx.enter_context(tc.tile_pool(name="ps_sc", bufs=2, space="PSUM"))
    ps_av = ctx.enter_context(tc.tile_pool(name="ps_av", bufs=3, space="PSUM"))
    ps_rs = ctx.enter_context(tc.tile_pool(name="ps_rs", bufs=1, space="PSUM"))

    # ---- gpsimd queue: tiny bias loads only (SWDGE is slow) ----
    nc.gpsimd.dma_start(bq_sb, bq.rearrange("(p r) -> p r", p=P))
    nc.gpsimd.dma_start(bk_sb, bk.rearrange("(p r) -> p r", p=P))
    _bcast_row_load(nc, bv_sb, bv)
    _bcast_row_load(nc, gam_sb, ln_g)
    _bcast_row_load(nc, bet_sb, ln_b)

    # ---- fp32 chunk loads (program order per engine = queue order) ----
    def nat_tile(name, shape=None):
        return znat.tile(shape or [P, GPC, D], F32, tag="znat", name=name)

    w_nat = {k: nat_tile(f"w_nat_{k}", [P, DC, D]) for k in "kqv"}
    zk_nat = [nat_tile(f"zk_nat{c}") for c in range(NT)]
    zq0_nat = nat_tile("zq0_nat")
    zv_nat = [nat_tile(f"zv_nat{c}") for c in range(NT)]
    zq_nat = {c: nat_tile(f"zq_nat{c}") for c in (1, 2, 3)}

    # scalar queue: weights + z_q0 + z_k tail (this queue also runs the
    # zk/w XBAR transposes, so keep its load list short)
    nc.scalar.dma_start(w_nat["k"], wr["k"])
    nc.scalar.dma_start(w_nat["q"], wr["q"])
    nc.scalar.dma_start(w_nat["v"], wr["v"])
    nc.scalar.dma_start(zq0_nat, zr_q[:, 0])
    nc.scalar.dma_start(zk_nat[2], zr_k[:, 2])
    nc.scalar.dma_start(zk_nat[3], zr_k[:, 3])
    # sync queue: z_k head + all of z_v + z_q tail
    nc.sync.dma_start(zk_nat[0], zr_k[:, 0])
    nc.sync.dma_start(zk_nat[1], zr_k[:, 1])
    nc.sync.dma_start(zv_nat[0], zr_v[:, 0])
    nc.sync.dma_start(zv_nat[1], zr_v[:, 1])
    nc.sync.dma_start(zv_nat[2], zr_v[:, 2])
    nc.sync.dma_start(zv_nat[3], zr_v[:, 3])
    nc.sync.dma_start(zq_nat[1], zr_q[:, 1])
    nc.sync.dma_start(zq_nat[2], zr_q[:, 2])
    nc.sync.dma_start(zq_nat[3], zr_q[:, 3])

    # ---- helpers ----
    def cast_chunk(nat, name, long=False, shape=None):
        pool = z16l if long else z16p
        t = pool.tile(shape or [P, GPC, D], BF16,
                      tag=(None if long else "z16c"), name=name)
        nc.vector.tensor_copy(t, nat)
        return t

    def xbar_rows(eng, z16c, zt_dst, c):
        # one XBAR transpose per packed row r: [128, 512] -> [d, 4, 128]
        for r in range(GPC):
            eng.dma_start(
                zt_dst[:, :, ds(c * NQ + r * P, P)], z16c[:, r, :], transpose=True
            )

    def pe_transpose_wv(w16v):
        # wt_v[d, do, e] with NATURAL e: strided copies undo the r-packing
        for do in range(DC):
            pt = ps_tp.tile([P, DC, P], BF16, tag="tp", name="pt")
            for r in range(DC):
                nc.tensor.transpose(pt[:, r, :], w16v[:, r, ts(do, P)], ident16)
            for r in range(DC):
                nc.vector.tensor_copy(wt["v"][:, do, r::DC], pt[:, r, :])

    def proj_sn(zt, wtk, bias_sb, dst, sn):
        for eo in range(DC):
            ps = ps_sc.tile([P, NQ], F32, tag="sc", name="proj_ps")
            for do in range(DC):
                nc.tensor.matmul(
                    ps,
                    wtk[:, do, ts(eo, P)],
                    zt[:, do, ts(sn, NQ)],
                    start=(do == 0),
                    stop=(do == DC - 1),
                )
            nc.vector.tensor_scalar_add(
                dst[:, eo, ts(sn, NQ)], ps, bias_sb[:, eo : eo + 1]
            )

    # ---- phase 1: casts in expected landing order (the in-order vector
    # stream must never hold a PE-dependent op before a cast), with each
    # XBAR transpose emitted right after its cast ----
    ztq0 = ztqp.tile([P, DC, NQ], BF16, tag="ztq", name="ztq0")
    wk16 = cast_chunk(w_nat["k"], "wk16", shape=[P, DC, D])
    xbar_rows(nc.scalar, wk16, wt["k"], 0)
    zk16_0 = cast_chunk(zk_nat[0], "zk16_0")
    xbar_rows(nc.scalar, zk16_0, zt_k, 0)
    wq16 = cast_chunk(w_nat["q"], "wq16", shape=[P, DC, D])
    xbar_rows(nc.scalar, wq16, wt["q"], 0)
    zk16_1 = cast_chunk(zk_nat[1], "zk16_1")
    xbar_rows(nc.scalar, zk16_1, zt_k, 1)
    wv16 = cast_chunk(w_nat["v"], "wv16", long=True, shape=[P, DC, D])
    zv16_0 = cast_chunk(zv_nat[0], "zv16_0")
    xbar_rows(nc.sync, zv16_0, zt_v, 0)
    zq16_0 = cast_chunk(zq0_nat, "zq16_0")
    xbar_rows(nc.scalar, zq16_0, ztq0, 0)
    zv16_1 = cast_chunk(zv_nat[1], "zv16_1")
    xbar_rows(nc.sync, zv16_1, zt_v, 1)
    zk16_2 = cast_chunk(zk_nat[2], "zk16_2")
    xbar_rows(nc.scalar, zk16_2, zt_k, 2)
    zv16_2 = cast_chunk(zv_nat[2], "zv16_2")
    xbar_rows(nc.sync, zv16_2, zt_v, 2)
    zk16_3 = cast_chunk(zk_nat[3], "zk16_3")
    xbar_rows(nc.scalar, zk16_3, zt_k, 3)
    zv16_3 = cast_chunk(zv_nat[3], "zv16_3")
    xbar_rows(nc.sync, zv16_3, zt_v, 3)

    qts = {}
    zq16s = {}

    def project_q0():
        qt = qtp.tile([P, DC, NQ], BF16, tag="qt", name="qt0")
        for eo in range(DC):
            ps = ps_sc.tile([P, NQ], F32, tag="sc", name="q0_ps")
            for do in range(DC):
                nc.tensor.matmul(
                    ps, wt["q"][:, do, ts(eo, P)], ztq0[:, do, :],
                    start=(do == 0), stop=(do == DC - 1),
                )
            nc.vector.tensor_scalar_add(qt[:, eo, :], ps, bq_sb[:, eo : eo + 1])
        qts[0] = qt

    expp_tiles = {}

    def scores_chunks(tq, skcs):
        expT = expp_tiles[tq]
        qt = qts[tq]
        for skc in skcs:
            pss = ps_sc.tile([P, NQ], F32, tag="sc", name="pss")
            for eo in range(DC):
                nc.tensor.matmul(
                    pss, Kt[:, eo, ts(skc, P)], qt[:, eo, :],
                    start=(eo == 0), stop=(eo == DC - 1),
                )
            nc.scalar.activation(
                expT[:, skc, :], pss,
                mybir.ActivationFunctionType.Exp,
                scale=inv_sqrt_d,
            )

    # PE order: K proj slices chase the XBARs, then Q0, then scores0
    proj_sn(zt_k, wt["k"], bk_sb, Kt, 0)
    project_q0()
    proj_sn(zt_k, wt["k"], bk_sb, Kt, 1)
    proj_sn(zt_k, wt["k"], bk_sb, Kt, 2)
    proj_sn(zt_k, wt["k"], bk_sb, Kt, 3)
    expp_tiles[0] = expp.tile([P, SC, NQ], BF16, tag="expT", name="expT0")
    scores_chunks(0, range(SC))

    def v_projection():
        pe_transpose_wv(wv16)
        for sko in range(SC):
            ps = ps_av.tile([P, D], F32, tag="av", name="vproj_ps")
            for do in range(DC):
                nc.tensor.matmul(
                    ps, zt_v[:, do, ts(sko, P)], wt["v"][:, do, :],
                    start=(do == 0), stop=(do == DC - 1),
                )
            nc.vector.tensor_tensor(V[:, sko, :], ps, bv_sb, ADD)

    def project_q_slice(tq):
        # cast (if fp32-loaded), XBAR-transpose, then project on the PE
        if tq not in zq16s:
            zq16s[tq] = cast_chunk(zq_nat[tq], f"zq16_{tq}")
        ztq = ztqp.tile([P, DC, NQ], BF16, tag="ztq", name=f"ztq{tq}")
        xbar_rows(nc.sync, zq16s[tq], ztq, 0)
        qt = qtp.tile([P, DC, NQ], BF16, tag="qt", name=f"qt{tq}")
        for eo in range(DC):
            psq = ps_sc.tile([P, NQ], F32, tag="sc", name="psq")
            for do in range(DC):
                nc.tensor.matmul(
                    psq, wt["q"][:, do, ts(eo, P)], ztq[:, do, :],
                    start=(do == 0), stop=(do == DC - 1),
                )
            nc.vector.tensor_scalar_add(qt[:, eo, :], psq, bq_sb[:, eo : eo + 1])
        qts[tq] = qt

    def av_subtile(expT, tq, m):
        so = tq * NM + m
        pso = ps_av.tile([P, D], F32, tag="av", name="pso")
        psr = ps_rs.tile([P, 2], F32, tag="rs", name="psr")
        for skc in range(SC):
            lhsT = expT[:, skc, ts(m, P)]
            nc.tensor.matmul(
                pso, lhsT, V[:, skc, :], start=(skc == 0), stop=(skc == SC - 1)
            )
            nc.tensor.matmul(
                psr, lhsT, ones, start=(skc == 0), stop=(skc == SC - 1)
            )
        st6 = ep.tile([P, 6], F32, tag="st6", name="st6")
        nc.vector.bn_stats(st6, pso)
        nc.vector.bn_aggr(stats[:, so, :], st6)
        r_eps = ep.tile([P, 1], F32, tag="r_eps", name="r_eps")
        nc.vector.tensor_scalar(
            r_eps, psr[:, 0:1], float(LN_EPS), 0.0, op0=MUL, op1=ADD
        )
        nc.vector.tensor_scalar(
            tvals[:, so : so + 1], psr[:, 0:1], r_eps, stats[:, so, 1:2],
            op0=MUL, op1=ADD,
        )
        av_sb = otp.tile([P, D], F32, tag="ot", name=f"av{so}")
        nc.vector.tensor_copy(av_sb, pso)
        return av_sb

    def sqrt_batch(tq):
        msl = ds(tq * NM, NM)
        nc.scalar.activation(
            svals[:, msl], tvals[:, msl], mybir.ActivationFunctionType.Sqrt
        )
        nc.vector.reciprocal(svals[:, msl], svals[:, msl])

    def finish_subtile(av_sb, tq, m):
        so = tq * NM + m
        nc.vector.tensor_scalar(
            nms[:, so : so + 1], stats[:, so, 0:1], svals[:, so : so + 1], -1.0,
            op0=MUL, op1=MUL,
        )
        nc.vector.tensor_scalar(
            av_sb, av_sb, svals[:, so : so + 1], nms[:, so : so + 1],
            op0=MUL, op1=ADD,
        )
        nc.gpsimd.tensor_tensor(av_sb, av_sb, gam_sb, MUL)
        nc.gpsimd.tensor_tensor(av_sb, av_sb, bet_sb, ADD)
        out_eng = nc.sync if m % 2 == 0 else nc.scalar
        out_eng.dma_start(outr[:, tq, m, :], av_sb)

    # ---- phase 2: attention body ----
    v_projection()
    avs = [av_subtile(expp_tiles[0], 0, m) for m in range(NM)]
    project_q_slice(1)
    sqrt_batch(0)
    for m in range(NM):
        finish_subtile(avs[m], 0, m)

    for tq in range(1, NT):
        expp_tiles[tq] = expp.tile([P, SC, NQ], BF16, tag="expT", name=f"expT{tq}")
        scores_chunks(tq, range(SC))
        if tq + 1 < NT:
            project_q_slice(tq + 1)
        avs = [av_subtile(expp_tiles[tq], tq, m) for m in range(NM)]
        sqrt_batch(tq)
        for m in range(NM):
            finish_subtile(avs[m], tq, m)

    ctx.close()


_NC_CACHE = None


def _build():
    global _NC_CACHE
    if _NC_CACHE is not None:
        return _NC_CACHE
    nc = bacc.Bacc("TRN2", target_bir_lowering=False, debug=False, num_devices=B)
    shapes = {
        "z_q": [S, D], "z_k": [S, D], "z_v": [S, D],
        "Wq": [D, D], "Wk": [D, D], "Wv": [D, D],
        "bq": [D], "bk": [D], "bv": [D],
        "ln_gamma": [D], "ln_beta": [D],
    }
    ins = {
        k: nc.dram_tensor(k, shapes[k], F32, kind="ExternalInput").ap()
        for k in INPUT_NAMES
    }
    out = nc.dram_tensor("out", [S, D], F32, kind="ExternalOutput").ap()
    with tile.TileContext(nc) as tc:
        _build_tile_kernel(tc, ins, out)
    nc.compile()
    _NC_CACHE = nc
    return nc


def _run(inputs, **spmd_kwargs):
    nc = _build()
    arrs = {k: np.ascontiguousarray(np.asarray(inputs[k]), dtype=np.float32)
            for k in INPUT_NAMES}
    in_maps = []
    for b in range(B):
        m = {"z_q": arrs["z_q"][b], "z_k": arrs["z_k"][b], "z_v": arrs["z_v"][b]}
        for k in ("Wq", "bq", "Wk", "bk", "Wv", "bv", "ln_gamma", "ln_beta"):
            m[k] = arrs[k]
        in_maps.append(m)
    res = run_bass_kernel_spmd(nc, in_maps, core_ids=list(range(B)), **spmd_kwargs)
    out = np.stack([res.results[b]["out"] for b in range(B)], axis=0)
    return out, res


def kernel(**inputs):
    out, _ = _run(inputs)
    return out


# revision 16
# speedup vs baseline: 1.4229x; 1.3665x over previous
"""Trainium2 Bass kernel for nn_CrossAttention (B=8, S=2048, D=512, fp32).

Sharding: data-parallel over batch across the 8 NeuronCores (one batch
element per core); the 512x512 projection weights are replicated.

Per-core dataflow (matmul inputs in bf16, fp32 PSUM accumulation):
  1. PE-transpose z/W fp32 blocks (cast to bf16 on the PSUM->SBUF copy) so
     the contraction dim (d) lands on partitions.
  2. Projections:  Qt[e,sq] = WqT.T @ zqT (+bq), Kt[e,sk] likewise,
                   V[sk,e]  = zvT.T @ WvT (+bv)
  3. Per 512-wide sq macro-tile:
       scoresT[sk, sq] = Kt.T @ Qt  (PSUM fp32) -> exp(./sqrt(D)) -> bf16
       per 128-row sq subtile:
         AV   psum[sq,e]  = sum_skc expT_chunk.T @ V_chunk
         rsum psum[sq,2]  = sum_skc expT_chunk.T @ ones
         ot = AV * (1/rsum); bn_stats/aggr -> stash mean/var
  4. Single batched Sqrt+reciprocal for all row stats (one ACT table load),
     then per subtile: (ot-mu)*rstd*gamma+beta -> DMA out.
  Softmax skips the max-subtraction: scores ~ N(0,1), so exp() is safely
  within fp32 range; matches jax softmax up to rounding.
"""

import math
import os
import sys
from contextlib import ExitStack

for _p in ("/opt/trn_rl_repo", "/root/.axon_site/_ro/trn_rl_repo"):
    if os.path.isdir(_p) and _p not in sys.path:
        sys.path.append(_p)

import numpy as np

import concourse.bacc as bacc
import concourse.bass as bass
import concourse.mybir as mybir
import concourse.tile as tile
from concourse.bass import ds, ts
from concourse.bass_utils import run_bass_kernel_spmd
from concourse.masks import make_identity

P = 128
B = 8
S = 2048
D = 512
DC = D // P       # 4   chunks of the model dim
SC = S // P       # 16  chunks of the sequence dim
NQ = 512          # sq macro-tile width (matmul free dim)
NT = S // NQ      # 4   macro tiles
LN_EPS = 1e-5
F32 = mybir.dt.float32
BF16 = mybir.dt.bfloat16

INPUT_NAMES = (
    "z_q", "z_k", "z_v", "Wq", "bq", "Wk", "bk", "Wv", "bv",
    "ln_gamma", "ln_beta",
)


def _bcast_row_load(nc, dst, src_1d):
    """DMA-replicate a [D] DRAM vector across all partitions of dst [P, D]."""
    src = bass.AP(
        tensor=src_1d.tensor,
        offset=src_1d.offset,
        ap=[[0, dst.shape[0]]] + list(src_1d.ap),
    )
    nc.gpsimd.dma_start(out=dst, in_=src)


def _build_tile_kernel(tc, ins, out):
    nc = tc.nc
    z_q, z_k, z_v, Wq, bq, Wk, bk, Wv, bv, ln_g, ln_b = (ins[k] for k in INPUT_NAMES)

    ctx = ExitStack()
    singles = ctx.enter_context(tc.tile_pool(name="singles", bufs=1))

    ident = singles.tile([P, P], F32)
    make_identity(nc, ident)
    ident16 = singles.tile([P, P], BF16)
    nc.vector.tensor_copy(ident16, ident)

    Qt = singles.tile([P, DC, S], BF16)   # [e_in, e_out, sq]
    Kt = singles.tile([P, DC, S], BF16)   # [e_in, e_out, sk]
    V = singles.tile([P, SC, D], BF16)    # [sk_in, sk_out, e]

    # ------------- phase 1: PE transposes (cast to bf16) + projections ------
    inv_sqrt_d = 1.0 / math.sqrt(D)
    outr = out.rearrange("(so p) d -> p so d", p=P)
    zt_q = singles.tile([P, DC, S], BF16)  # q stays live through phase 2
    with (
        tc.tile_pool(name="wz", bufs=3) as wz,
        tc.tile_pool(name="wp", bufs=2) as wp,
        tc.tile_pool(name="ztp", bufs=2) as ztp,
        tc.tile_pool(name="ps_tp", bufs=5, space="PSUM") as ps_tp,
        tc.tile_pool(name="ps13", bufs=3, space="PSUM") as ps13,
    ):
        def transpose_w(W, name):
            # wt[d_in, d_out, e] bf16, via PE transpose of bf16 blocks
            w_nat = wp.tile([P, DC, D], F32, tag="wnat", name="w_nat")
            nc.sync.dma_start(w_nat, W.rearrange("(eo p) d -> p eo d", p=P))
            w_n16 = wp.tile([P, DC, D], BF16, tag="wnat16", name="w_n16")
            nc.vector.tensor_copy(w_n16, w_nat)
            wt = singles.tile([P, DC, D], BF16, tag=f"wt_{name}", name=f"wt_{name}")
            for do in range(DC):
                pt = ps_tp.tile([P, DC, P], BF16, tag="tp", name="pt")
                for eo in range(DC):
                    nc.tensor.transpose(
                        pt[:, eo, :], w_n16[:, eo, ts(do, P)], ident16
                    )
                nc.vector.tensor_copy(wt[:, do, :], pt)
            return wt

        def transpose_z(z, zt, engines=(None, None)):
            # zt[d_in, d_out, s] bf16; ladder-sized chunks (small first so the
            # PE starts early), loads+casts emitted ahead of the transposes.
            sizes = (1, 1, 2, 4, 4, 4)   # 128-row groups per chunk, sum = 16
            zr = z.rearrange("(g p) d -> p g d", p=P)
            zn16s = []
            g0 = 0
            for c, jc in enumerate(sizes):
                znat = wz.tile([P, 4, D], F32, tag="znat", name="znat")[:, :jc]
                eng = engines[c % 2] or (nc.scalar if c % 2 == 0 else nc.sync)
                eng.dma_start(znat, zr[:, ds(g0, jc), :])
                zn16 = wz.tile([P, 4, D], BF16, tag="zn16", name="zn16")[:, :jc]
                nc.vector.tensor_copy(zn16, znat)
                zn16s.append((g0, jc, zn16))
                g0 += jc
            for g0, jc, zn16 in zn16s:
                for do in range(DC):
                    pt = ps_tp.tile([P, 4, P], BF16, tag="tp", name="pt")[:, :jc]
                    for j in range(jc):
                        nc.tensor.transpose(
                            pt[:, j, :], zn16[:, j, ts(do, P)], ident16
                        )
                    nc.vector.tensor_copy(zt[:, do, ds(g0 * P, jc * P)], pt)
            return zt

        def project_qk(zt, wt, bias_sb, dst, sn_range):
            # dst[e, s] = W @ z.T + b   laid out [P, DC(e_out), S]
            for sn in sn_range:
                for eo in range(DC):
                    ps = ps13.tile([P, NQ], F32, tag="proj")
                    for do in range(DC):
                        nc.tensor.matmul(
                            ps,
                            wt[:, do, ts(eo, P)],
                            zt[:, do, ts(sn, NQ)],
                            start=(do == 0),
                            stop=(do == DC - 1),
                        )
                    nc.vector.tensor_scalar_add(
                        dst[:, eo, ts(sn, NQ)], ps, bias_sb[:, eo : eo + 1]
                    )

        # K first (scores need all of Kt), then V, then Q fused into phase 2.
        zt_k = ztp.tile([P, DC, S], BF16, tag="zt", name="zt_k")
        transpose_z(z_k, zt_k)
        wt_k = transpose_w(Wk, "k")

        # small constants / biases: gpsimd SWDGE, off the hot HWDGE queues
        ones_f32 = singles.tile([P, 2], F32)
        nc.vector.memset(ones_f32, 1.0)
        ones = singles.tile([P, 2], BF16)
        nc.vector.tensor_copy(ones, ones_f32)
        eps_sb = singles.tile([P, 1], F32)
        nc.vector.memset(eps_sb, LN_EPS)
        bq_sb = singles.tile([P, DC], F32)
        nc.gpsimd.dma_start(bq_sb, bq.rearrange("(eo p) -> p eo", p=P))
        bk_sb = singles.tile([P, DC], F32)
        nc.gpsimd.dma_start(bk_sb, bk.rearrange("(eo p) -> p eo", p=P))
        bv_sb = singles.tile([P, D], F32)
        _bcast_row_load(nc, bv_sb, bv)
        gam_sb = singles.tile([P, D], F32)
        _bcast_row_load(nc, gam_sb, ln_g)
        bet_sb = singles.tile([P, D], F32)
        _bcast_row_load(nc, bet_sb, ln_b)

        project_qk(zt_k, wt_k, bk_sb, Kt, range(S // NQ))

        zt_v = ztp.tile([P, DC, S], BF16, tag="zt", name="zt_v")
        transpose_z(z_v, zt_v)
        wt_v = transpose_w(Wv, "v")
        # V[sk, e] = z_v @ Wv.T + bv   laid out [P, SC(sk_out), D]
        for sko in range(SC):
            ps = ps13.tile([P, D], F32, tag="proj")
            for do in range(DC):
                nc.tensor.matmul(
                    ps,
                    zt_v[:, do, ts(sko, P)],
                    wt_v[:, do, :],
                    start=(do == 0),
                    stop=(do == DC - 1),
                )
            nc.vector.tensor_tensor(V[:, sko, :], ps, bv_sb, mybir.AluOpType.add)

        transpose_z(z_q, zt_q)
        wt_q = transpose_w(Wq, "q")

    # ---------------- phase 2: Q-projection + attention + layernorm ---------
    with (
        tc.tile_pool(name="expp", bufs=3) as expp,
        tc.tile_pool(name="otp", bufs=8) as otp,
        tc.tile_pool(name="ep", bufs=4) as ep,
        tc.tile_pool(name="ps_sc", bufs=3, space="PSUM") as ps_sc,
        tc.tile_pool(name="ps_av", bufs=3, space="PSUM") as ps_av,
        tc.tile_pool(name="ps_rs", bufs=2, space="PSUM") as ps_rs,
    ):
        stats = singles.tile([P, SC, 2], F32)   # per-subtile (mean, var)
        rstd_all = singles.tile([P, SC], F32)
        def project_q_slice(tq):
            for eo in range(DC):
                psq = ps_sc.tile([P, NQ], F32, tag="sc", name="psq")
                for do in range(DC):
                    nc.tensor.matmul(
                        psq,
                        wt_q[:, do, ts(eo, P)],
                        zt_q[:, do, ts(tq, NQ)],
                        start=(do == 0),
                        stop=(do == DC - 1),
                    )
                nc.vector.tensor_scalar_add(
                    Qt[:, eo, ts(tq, NQ)], psq, bq_sb[:, eo : eo + 1]
                )

        ots = []
        project_q_slice(0)
        for tq in range(NT):
            expT = expp.tile([P, SC, NQ], BF16, tag="expT")  # [sk_in,sk_out,sq]
            for skc in range(SC):
                pss = ps_sc.tile([P, NQ], F32, tag="sc")
                for eo in range(DC):
                    nc.tensor.matmul(
                        pss,
                        Kt[:, eo, ts(skc, P)],
                        Qt[:, eo, ts(tq, NQ)],
                        start=(eo == 0),
                        stop=(eo == DC - 1),
                    )
                nc.scalar.activation(
                    expT[:, skc, :], pss,
                    mybir.ActivationFunctionType.Exp,
                    scale=inv_sqrt_d,
                )
            if tq + 1 < NT:
                project_q_slice(tq + 1)
            for m in range(NQ // P):
                so = tq * (NQ // P) + m  # global 128-row subtile index
                pso = ps_av.tile([P, D], F32, tag="av")
                psr = ps_rs.tile([P, 2], F32, tag="rs")
                for skc in range(SC):
                    lhsT = expT[:, skc, ts(m, P)]
                    nc.tensor.matmul(
                        pso, lhsT, V[:, skc, :],
                        start=(skc == 0), stop=(skc == SC - 1),
                    )
                    nc.tensor.matmul(
                        psr, lhsT, ones,
                        start=(skc == 0), stop=(skc == SC - 1),
                    )
                rinv = ep.tile([P, 1], F32, tag="rinv")
                nc.vector.reciprocal(rinv, psr[:, 0:1])
                ot = otp.tile([P, D], F32, tag="ot")
                nc.vector.tensor_scalar_mul(ot, pso, rinv)
                st6 = ep.tile([P, 6], F32, tag="st6")
                nc.vector.bn_stats(st6, ot)
                nc.vector.bn_aggr(stats[:, so, :], st6)
                ots.append(ot)
            # per-macro-tile epilogue: batch Sqrt over this tile's 4 subtiles
            mslice = ds(tq * (NQ // P), NQ // P)
            nc.scalar.activation(
                rstd_all[:, mslice], stats[:, mslice, 1],
                mybir.ActivationFunctionType.Sqrt,
                bias=eps_sb,
            )
            nc.vector.reciprocal(rstd_all[:, mslice], rstd_all[:, mslice])
            for m in range(NQ // P):
                so = tq * (NQ // P) + m
                ot = ots[so]
                nc.vector.tensor_scalar(
                    ot, ot, stats[:, so, 0:1], rstd_all[:, so : so + 1],
                    op0=mybir.AluOpType.subtract,
                    op1=mybir.AluOpType.mult,
                )
                nc.vector.tensor_tensor(ot, ot, gam_sb, mybir.AluOpType.mult)
                nc.vector.tensor_tensor(ot, ot, bet_sb, mybir.AluOpType.add)
                nc.sync.dma_start(outr[:, so, :], ot)
    ctx.close()


_NC_CACHE = None


def _build():
    global _NC_CACHE
    if _NC_CACHE is not None:
        return _NC_CACHE
    nc = bacc.Bacc("TRN2", target_bir_lowering=False, debug=False, num_devices=B)
    shapes = {
        "z_q": [S, D], "z_k": [S, D], "z_v": [S, D],
        "Wq": [D, D], "Wk": [D, D], "Wv": [D, D],
        "bq": [D], "bk": [D], "bv": [D],
        "ln_gamma": [D], "ln_beta": [D],
    }
    ins = {
        k: nc.dram_tensor(k, shapes[k], F32, kind="ExternalInput").ap()
        for k in INPUT_NAMES
    }
    out = nc.dram_tensor("out", [S, D], F32, kind="ExternalOutput").ap()
    with tile.TileContext(nc) as tc:
        _build_tile_kernel(tc, ins, out)
    nc.compile()
    _NC_CACHE = nc
    return nc


def _run(inputs, **spmd_kwargs):
    nc = _build()
    arrs = {k: np.ascontiguousarray(np.asarray(inputs[k]), dtype=np.float32)
            for k in INPUT_NAMES}
    in_maps = []
    for b in range(B):
        m = {"z_q": arrs["z_q"][b], "z_k": arrs["z_k"][b], "z_v": arrs["z_v"][b]}
        for k in ("Wq", "bq", "Wk", "bk", "Wv", "bv", "ln_gamma", "ln_beta"):
            m[k] = arrs[k]
        in_maps.append(m)
    res = run_bass_kernel_spmd(nc, in_maps, core_ids=list(range(B)), **spmd_kwargs)
    out = np.stack([res.results[b]["out"] for b in range(B)], axis=0)
    return out, res


def kernel(**inputs):
    out, _ = _run(inputs)
    return out

